# revision 28
# baseline (speedup 1.0000x reference)
"""Trainium2 Bass kernel for nn_AbsoluteThresholdTokenPruner.

Reference math (per batch b):
  headsum[s, k]   = sum_h probs[b, s, h, k]        (row masked to 0 if attention_mask[b,s] < 0)
  global_sum[b,g] = sum_s headsum[s, g]            for g in [0, 64)
  local_sum[b, j] = sum_{s,c: s+c-256=j} headsum[s, 64+c]   for c in [0, 513)
  probs_sum       = local_sum  (+ global_sum scattered onto j ∈ [0,64) via index arrays)
  scores          = probs_sum / max_j(probs_sum)
  new_mask        = where(scores < max(1e-5, thr), -10000, 0)

Device strategy (8 cores, data parallel over (batch, seq)):
  Core c handles batch b = c // 4, rows s ∈ [1024*(c%4), 1024*(c%4)+1024).
  Per 128-row block, the local window probs are loaded from HBM into SBUF with a
  *skewed* access pattern produced on the DRAM-read side: partition p reads its
  row starting p elements early (stride RS-1 across partitions), so the tile
  holds dest (p, t) = row_p[t - p]. Summing heads with strided adds preserves
  the skew, after which the banded anti-diagonal sum over sources collapses to
  a plain partition-dim reduction: a matmul whose stationary operand is the
  per-row attention-mask gate (1/0), which applies the row masking for free.
  Column t of the reduced tile holds the contribution to output key
  j = s0 + 128*blk + t - 256. The global columns are reduced by 12 accumulating
  matmuls on the tensor engine. Cross-block/cross-core overlap-add of the
  per-block [1, 640] partial sums, the tiny global-sum scatter, the per-batch
  max, and the threshold are done on host (O(B*S) work vs O(B*S*H*577) on
  device).
"""

import numpy as np

import concourse.bass as bass
import concourse.bacc as bacc
import concourse.mybir as mybir
from concourse.tile import TileContext
from concourse.bass_utils import run_bass_kernel_spmd

F32 = mybir.dt.float32

B, S, H, G, W = 2, 4096, 12, 64, 513
HALF = W // 2               # 256
NCORE = 8
SC = (B * S) // NCORE       # 1024 rows per core
PB = 128                    # rows per block (SBUF partitions)
NBLK = SC // PB             # 8 blocks per core
RS = H * W                  # 6156 local elems per row
AW = RS + PB - 1            # 6283 skewed tile width
T1W = RS // 2 + PB - 1      # 3205
T2W = RS // 4 + PB - 1      # 1666
GLW = H * G                 # 768
LOW = W + PB - 1            # 640 local output columns per block
OW = LOW + G                # 704 = 640 local + 64 global output columns
NABUF = 4                   # skewed-tile buffers (DMA/compute overlap)

_NC_CACHE = None
LAST_RESULTS = None         # BassKernelResults of the most recent run


def _build_bass():
    nc = bacc.Bacc("TRN2", target_bir_lowering=False, num_devices=NCORE)

    pl = nc.dram_tensor("pl", [SC, H, W], F32, kind="ExternalInput")
    pg = nc.dram_tensor("pg", [SC, H, G], F32, kind="ExternalInput")
    mind = nc.dram_tensor("mind", [SC, 1], F32, kind="ExternalInput")
    band = nc.dram_tensor("band", [PB, LOW], F32, kind="ExternalInput")
    outp = nc.dram_tensor("outp", [NBLK, OW], F32, kind="ExternalOutput")

    with TileContext(nc) as tc:
        # Persistent skewed tiles. The skew is produced on the *DRAM read* side
        # (flat addresses; per-partition SBUF write offsets are not honored by
        # the DMA beyond 16B granularity): partition p reads AW contiguous
        # elems starting at flat offset p*(RS-1), so dest (p, t) = row_p[t-p].
        # Cells with t-p outside [0, RS) hold neighboring-row garbage, and
        # out-of-band in-row positions mix misaligned heads; the band multiply
        # zeroes everything outside the window band before the reduction.
        a_bufs = [nc.alloc_sbuf_tensor(f"askew{i}", [PB, AW], F32) for i in range(NABUF)]

        with tc.tile_pool(name="const", bufs=1) as cpool, \
             tc.tile_pool(name="work", bufs=2) as pool, \
             tc.tile_pool(name="psum", bufs=2, space="PSUM") as pspool:
            bandt = cpool.tile([PB, LOW], F32)
            nc.sync.dma_start(out=bandt[:, :], in_=band[:, :])

            for blk in range(NBLK):
                a = a_bufs[blk % NABUF]
                # Skewed load via overlapping DRAM reads, issued as two
                # partition-half transfers so each queue interleaves two
                # in-flight descriptors (hides per-dma re-arm gaps).
                HPB = PB // 2
                nc.sync.dma_start(
                    out=bass.AP(a, 0, [[AW, HPB], [1, AW]]),
                    in_=bass.AP(pl, blk * PB * RS, [[RS - 1, HPB], [1, AW]]),
                )
                nc.sync.dma_start(
                    out=bass.AP(a, HPB * AW, [[AW, HPB], [1, AW]]),
                    in_=bass.AP(pl, (blk * PB + HPB) * RS - HPB, [[RS - 1, HPB], [1, AW]]),
                )
                bt = pool.tile([PB, GLW], F32, tag="bt")
                nc.sync.dma_start(
                    out=bt[:, :],
                    in_=bass.AP(pg, blk * PB * GLW, [[GLW, PB], [1, GLW]]),
                )
                mt = pool.tile([PB, 1], F32, tag="mt")
                nc.sync.dma_start(out=mt[:, :], in_=mind[blk * PB:(blk + 1) * PB, :])

                # Head reduction (skew-preserving): offsets 513*h via 2/2/3 tree.
                t1 = pool.tile([PB, T1W], F32, tag="t1")
                # First tree level runs on the otherwise-idle GpSimd engine,
                # except for the first/last block where DVE (idle during
                # ramp/drain and ~1.5x faster) shortens the critical path.
                t1_eng = nc.vector if blk in (0, NBLK - 1) else nc.gpsimd
                t1_eng.tensor_add(out=t1[:, :], in0=a[:, 0:T1W], in1=a[:, RS // 2:RS // 2 + T1W])
                t2 = pool.tile([PB, T2W], F32, tag="t2")
                nc.vector.tensor_add(out=t2[:, :], in0=t1[:, 0:T2W], in1=t1[:, RS // 4:RS // 4 + T2W])

                pa = pool.tile([PB, LOW], F32, tag="pa")
                nc.vector.tensor_add(out=pa[:, :], in0=t2[:, 0:LOW], in1=t2[:, W:W + LOW])
                pb_ = pool.tile([PB, LOW], F32, tag="pb")
                nc.vector.tensor_add(out=pb_[:, :], in0=pa[:, :], in1=t2[:, 2 * W:2 * W + LOW])

                # Zero the out-of-band garbage.
                pc = pool.tile([PB, LOW], F32, tag="pc")
                nc.vector.tensor_mul(out=pc[:, :], in0=pb_[:, :], in1=bandt[:, :])

                # Partition-dim reduction with the mask gate as the stationary
                # operand: colsum[t] = sum_p mind[p] * pc[p, t].
                ps = pspool.tile([1, LOW], F32, tag="ps")
                nc.tensor.matmul(ps[0:1, 0:512], mt[:, :], pc[:, 0:512], start=True, stop=True)
                nc.tensor.matmul(ps[0:1, 512:LOW], mt[:, :], pc[:, 512:LOW], start=True, stop=True)

                # Global columns: accumulate the per-head [128, 64] blocks on
                # the tensor engine (contract over rows, gated by mind).
                psg = pspool.tile([1, G], F32, tag="psg")
                for h in range(H):
                    nc.tensor.matmul(psg[0:1, :], mt[:, :], bt[:, h * G:(h + 1) * G],
                                     start=(h == 0), stop=(h == H - 1))

                # Trigger the store from the Scalar engine so the Sync
                # sequencer's in-order stream stays pure loads (no
                # head-of-line blocking of the next block's big load).
                osb = pool.tile([1, OW], F32, tag="osb")
                nc.scalar.copy(out=osb[:, 0:LOW], in_=ps[:, :])
                nc.scalar.copy(out=osb[:, LOW:OW], in_=psg[:, :])
                nc.scalar.dma_start(out=outp[blk:blk + 1, :], in_=osb[:, :])

    nc.compile()
    return nc


def _band_array():
    band = np.zeros((PB, LOW), np.float32)
    for p in range(PB):
        band[p, p:p + W] = 1.0
    return band


def _get_nc():
    global _NC_CACHE
    if _NC_CACHE is None:
        _NC_CACHE = _build_bass()
    return _NC_CACHE


def kernel(attention_mask, attention_probs, keep_threshold,
           max_num_global_attn_indices, loc_b, loc_i, glob_b, glob_i):
    attention_mask = np.asarray(attention_mask, dtype=np.float32)
    attention_probs = np.asarray(attention_probs, dtype=np.float32)
    thr_in = float(np.asarray(keep_threshold, dtype=np.float32).reshape(-1)[0])
    gn = int(np.asarray(max_num_global_attn_indices).reshape(-1)[0])
    loc_b = np.asarray(loc_b).astype(np.int64)
    loc_i = np.asarray(loc_i).astype(np.int64)
    glob_b = np.asarray(glob_b).astype(np.int64)
    glob_i = np.asarray(glob_i).astype(np.int64)

    assert attention_probs.shape == (B, S, H, G + W), attention_probs.shape
    assert attention_mask.shape == (B, S)
    assert gn == G, gn

    band = _band_array()
    mind_full = (attention_mask >= 0).astype(np.float32)

    in_maps = []
    for c in range(NCORE):
        b = c // (NCORE // B)
        s0 = SC * (c % (NCORE // B))
        blk = attention_probs[b, s0:s0 + SC]
        in_maps.append({
            "pl": np.ascontiguousarray(blk[:, :, G:]),
            "pg": np.ascontiguousarray(blk[:, :, :G]),
            "mind": np.ascontiguousarray(mind_full[b, s0:s0 + SC]).reshape(SC, 1),
            "band": band,
        })

    nc = _get_nc()
    res = run_bass_kernel_spmd(nc, in_maps, core_ids=list(range(NCORE)))
    global LAST_RESULTS
    LAST_RESULTS = res

    # Host finalize: overlap-add the per-block partial column sums.
    probs_sum = np.zeros((B, S), np.float32)
    gsum = np.zeros((B, G), np.float32)
    for c in range(NCORE):
        b = c // (NCORE // B)
        s0 = SC * (c % (NCORE // B))
        op = res.results[c]["outp"]
        for blk in range(NBLK):
            j0 = s0 + PB * blk - HALF
            lo = max(0, -j0)
            hi = min(LOW, S - j0)
            probs_sum[b, j0 + lo:j0 + hi] += op[blk, lo:hi]
            gsum[b] += op[blk, LOW:OW]

    global LAST_LOCAL_SUM, LAST_GSUM
    LAST_LOCAL_SUM = probs_sum.copy()
    LAST_GSUM = gsum.copy()
    np.add.at(probs_sum, (loc_b, loc_i), gsum[glob_b, glob_i])

    probs_max = probs_sum.max(axis=-1, keepdims=True)
    scores = (probs_sum / probs_max).astype(np.float32)
    thr = np.float32(max(1e-5, thr_in))
    new_attention_mask = np.where(scores < thr, np.float32(-10000.0), np.float32(0.0))
    return new_attention_mask, scores
